# revision 46
# baseline (speedup 1.0000x reference)
"""AlphaFold-style gated MSA attention on 8 Trainium2 NeuronCores.

Batch-sharded (128 batches -> 16 per core). Full inputs in, full output out.

Math per batch b (reference):
  q = (q_data @ Wq) * hk^-0.5          [Q, H, 32]
  k = m_data @ Wk ; v = m_data @ Wv    [K, H, 32]
  S[h] = q_h k_h^T + bias[b] + nb[h]   [H, Q, K]
  w = softmax(S, axis=-1)
  wa = w @ v                            [Q, H, 32]
  gate = sigmoid(q_data @ Wg + gb)
  out = (wa * gate).reshape(Q, 256) @ Wo + o_bias

Device-side formulation (per core, per batch):
  All tensors transposed (feature dim on partitions).  S^T[k, q] per head
  from kT/qT projections.  Softmax is unnormalized: the host precomputes
  pw[b, h] = exp(bias[b] + nb[h])^T in bf16 (one DMA per batch) so the
  per-head weights are a single DVE multiply:
      w^T = exp(S^T) * pw[h]
  The V-matmul uses lhsT = [v_h | 2.0] so PSUM row 32/96 accumulates
  2*sum_k w per head (softmax denominators for free).  Head pairs share a
  PSUM bank; evacuated pairs land in wa_all[:, pair, :], from which one
  strided DMA per batch gathers all 8 denominator rows and four per-pair
  DMAs rearrange heads into j-layout waA tiles for the output projection.
  Normalization and gating fuse:
      ga^T = waA * (1 + tanh(x/2 + gb/2)) * recip(2*sum)
  with the per-head recip broadcast across 32 partitions by an indicator
  matmul.  Output projection back to [q, 256] with o_bias added via a
  rank-1 fp8 hi/lo DoubleRow matmul; output stored bf16.
"""

import os
import sys

sys.path.insert(0, "/opt/trn_rl_repo")

import numpy as np
import ml_dtypes
from contextlib import ExitStack

import concourse.bass as bass  # noqa: F401  (engine types)
import concourse.bacc as bacc
import concourse.mybir as mybir
import concourse.tile as tile

BF16 = ml_dtypes.bfloat16
E4M3 = ml_dtypes.float8_e4m3fn

NUM_CORES = 8
B, Q, K, A = 128, 384, 384, 256
H, HD = 8, 32  # heads, head dim
OUT = 256
BPC = B // NUM_CORES  # batches per core


def _env(name, default):
    return os.environ.get(name, default)


def _build_body(ctx, tc, io, bpc):
    nc = tc.nc
    f32, bf = mybir.dt.float32, mybir.dt.bfloat16
    fp8 = mybir.dt.float8e4
    Exp = mybir.ActivationFunctionType.Exp
    Tanh = mybir.ActivationFunctionType.Tanh
    MUL, ADD = mybir.AluOpType.mult, mybir.AluOpType.add
    DR = mybir.MatmulPerfMode.DoubleRow

    const = ctx.enter_context(tc.tile_pool(name="const", bufs=1))
    lp = ctx.enter_context(tc.tile_pool(name="loads", bufs=int(_env("LP_BUFS", "3"))))
    pwp = ctx.enter_context(tc.tile_pool(name="pw", bufs=int(_env("PW_BUFS", "2"))))
    pp = ctx.enter_context(tc.tile_pool(name="proj", bufs=int(_env("PP_BUFS", "4"))))
    wp = ctx.enter_context(tc.tile_pool(name="work", bufs=int(_env("WP_BUFS", "4"))))
    wap = ctx.enter_context(tc.tile_pool(name="wa", bufs=int(_env("WA_BUFS", "2"))))
    gp = ctx.enter_context(tc.tile_pool(name="gating", bufs=int(_env("GP_BUFS", "3"))))
    outp = ctx.enter_context(tc.tile_pool(name="outp", bufs=3))
    # PSUM: 2 x 3 banks (S^T) + 2 x 1 bank (everything else) = 8 banks.
    Sp = ctx.enter_context(tc.tile_pool(name="psum_S", bufs=int(_env("S_BUFS", "2")), space="PSUM"))
    sp = ctx.enter_context(tc.tile_pool(name="psum_sm", bufs=int(_env("SP_BUFS", "2")), space="PSUM"))

    # ---- resident constants ----
    w_sb = {}
    for name in ("wq", "wk", "wv", "wg", "wo"):
        w_sb[name] = const.tile([128, 2, 256], bf, tag=name, name=name)
        nc.sync.dma_start(w_sb[name][:], io[name])
    # o_bias via fp8 hi/lo DoubleRow rank-1: [1,2,128] ones, [1,2,256] bias
    ones2 = const.tile([1, 2, 128], fp8, tag="ones2")
    nc.sync.dma_start(ones2[:], io["ones2"])
    obias2 = const.tile([1, 2, OUT], fp8, tag="obias2")
    nc.sync.dma_start(obias2[:], io["obias2"])
    gbh_sb = const.tile([128, 2], f32, tag="gbh")
    nc.sync.dma_start(gbh_sb[:], io["gbh"])
    if _env("ACT_WARM", "1") == "1":
        # dummy activation right after the first tiny const DMA: pulls the
        # ~1.3us ACT table load off the critical path (exp_and_others holds
        # both Exp and Tanh, so no further loads fire later)
        warm = const.tile([128, 2], f32, tag="warm")
        nc.scalar.activation(warm[:], gbh_sb[:], Exp)
    ind_sb = const.tile([4, 128], bf, tag="ind")
    nc.sync.dma_start(ind_sb[:], io["ind"])
    # duplicated identity for fp8 hi/lo DoubleRow adds of (bias+nb) into S
    idup = const.tile([128, 2, 128], fp8, tag="idup")
    nc.sync.dma_start(idup[:], io["idup"])

    # GPSIMD has no PSUM port: all psum evacuations go ACT or DVE.
    qk_ev = nc.scalar if _env("QK_EV", "dve") == "act" else nc.vector
    vv_ev = nc.scalar if _env("VV_EV", "dve") == "act" else nc.vector
    ob_ev = nc.scalar if _env("OB_EV", "dve") == "act" else nc.vector
    wa_ev = nc.scalar if _env("WA_EV", "dve") == "act" else nc.vector

    def ecopy(eng, dst, src):
        if eng is nc.scalar:
            eng.copy(dst, src)
        else:
            eng.tensor_copy(dst, src)

    from concourse.tile_rust import add_dep_helper as _adh

    wa_war = {}  # buffer-slot -> last strided sums DMA reading that slot
    n_wa_bufs = int(_env("WA_BUFS", "2"))
    states = {}

    def emit_loads(b, ld_pri=None):
        import contextlib
        hp = tc.high_priority() if ld_pri == 0 else contextlib.nullcontext()
        ld = lp.tile([128, 4, Q], bf, tag="ld", name=f"ld_{b}")
        with hp:
            nc.sync.dma_start(ld[:], io["inT"][b])
        cb = pwp.tile([128, H, 3, 2, Q], fp8, tag="cb", name=f"cb_{b}")
        nc.sync.dma_start(cb[:], io["cbT"][b])
        qT = pp.tile([128, 2, Q], bf, tag="qT", name=f"qT_{b}")  # [hc, j, q]
        kT = pp.tile([128, 2, Q], bf, tag="kT", name=f"kT_{b}")  # [hc, j, k]
        gt = pp.tile([128, 2, Q], bf, tag="gt", name=f"gt_{b}")
        # [k, kc, h*33+c | 2.0]; padded to 320 so every head has a 64-wide
        # lhsT window (M=64 writes initialized junk to PSUM rows 32-63,
        # letting the pair evacuation be one full-width copy).
        vv = pp.tile([128, 3, 320], bf, tag="vv", name=f"vv_{b}")
        sums_bf = [
            gp.tile([4, Q], bf, tag=f"sums_bf{i}", name=f"sums_bf{i}_{b}")
            for i in range(2)
        ]
        wa_all = wap.tile([128, 4, Q], bf, tag="wa_all", name=f"wa_all_{b}")
        waA = [
            pp.tile([128, Q], bf, tag=f"waA{j}", name=f"waA{j}_{b}") for j in range(2)
        ]
        states[b] = dict(
            ld=ld, cb=cb, qT=qT, kT=kT, gt=gt, vv=vv,
            sums_bf=sums_bf, wa_all=wa_all, waA=waA, psW=None, psO=None,
            wa_evs=[], ga=[None, None], ob=None, recb=[None, None],
            recb_tc=[None, None],
        )

    def proj_qkg(b, which, j):
        # which: "q" -> qT, "k" -> kT, "g" -> gt (tanh)
        st = states[b]
        ld = st["ld"]
        src_, wname = (ld[:, 0:2, :], "wq") if which == "q" else (
            (ld[:, 2:4, :], "wk") if which == "k" else (ld[:, 0:2, :], "wg")
        )
        ps = sp.tile([128, 512], f32, tag="sm", name=f"ps{which}{j}_{b}")
        for a in range(2):
            nc.tensor.matmul(
                ps[:, :Q],
                w_sb[wname][:, a, 128 * j : 128 * (j + 1)],
                src_[:, a, :],
                start=(a == 0),
                stop=(a == 1),
            )
        if which == "g":
            nc.scalar.activation(
                st["gt"][:, j, :], ps[:, :Q], Tanh, bias=gbh_sb[:, j : j + 1], scale=0.5
            )
        else:
            dst = st["qT"] if which == "q" else st["kT"]
            ecopy(qk_ev, dst[:, j, :], ps[:, :Q])

    def proj_v01(b):
        st = states[b]
        md = st["ld"][:, 2:4, :]
        vv = st["vv"]
        nc.gpsimd.memset(vv[:], 2.0)
        ps = sp.tile([128, 512], f32, tag="sm", name=f"psv01_{b}")
        for kc in range(2):
            for a in range(2):
                nc.tensor.matmul(
                    ps[:, 256 * kc : 256 * (kc + 1)],
                    md[:, a, 128 * kc : 128 * (kc + 1)],
                    w_sb["wv"][:, a, :],
                    start=(a == 0),
                    stop=(a == 1),
                )
        ecopy(
            vv_ev,
            vv[:, 0:2, 0:264].rearrange("p k (h c) -> p k h c", c=33)[:, :, :, 0:32],
            ps[:].rearrange("p (k h c) -> p k h c", k=2, c=32),
        )

    def proj_v2(b):
        st = states[b]
        md = st["ld"][:, 2:4, :]
        ps = sp.tile([128, 512], f32, tag="sm", name=f"psv2_{b}")
        for a in range(2):
            nc.tensor.matmul(
                ps[:, :256],
                md[:, a, 256:384],
                w_sb["wv"][:, a, :],
                start=(a == 0),
                stop=(a == 1),
            )
        ecopy(
            vv_ev,
            st["vv"][:, 2, 0:264].rearrange("p (h c) -> p h c", c=33)[:, :, 0:32],
            ps[:, :256].rearrange("p (h c) -> p h c", c=32),
        )

    PROJ_PIECES = [
        lambda b: proj_qkg(b, "q", 0),
        lambda b: proj_qkg(b, "k", 0),
        lambda b: proj_qkg(b, "q", 1),
        lambda b: proj_qkg(b, "k", 1),
        proj_v01,
        proj_v2,
        lambda b: proj_qkg(b, "g", 0),
        lambda b: proj_qkg(b, "g", 1),
    ]

    def emit_S(b, h):
        st = states[b]
        j, hh = h // 4, h % 4
        psS = Sp.tile([128, 1536], f32, tag="S", name=f"psS{h}_{b}")
        for kc in range(3):
            qk_mm = nc.tensor.matmul(
                psS[:, 512 * kc : 512 * kc + Q],
                st["kT"][32 * hh : 32 * (hh + 1), j, 128 * kc : 128 * (kc + 1)],
                st["qT"][32 * hh : 32 * (hh + 1), j, :],
                start=True,
                stop=False,
                tile_position=(32 * hh, 0),
            )
            # accumulate bias+nb via fp8 hi/lo DoubleRow identity-add
            add_mm = nc.tensor.matmul(
                psS[:, 512 * kc : 512 * kc + Q],
                idup[:],
                st["cb"][:, h, kc, :, :],
                start=False,
                stop=True,
                perf_mode=DR,
            )
            # no data dep links the group members; forbid scheduler reorder
            _adh(add_mm.ins, qk_mm.ins, reason="accumulation group order")
        sview = psS[:].rearrange("p (c x) -> p c x", x=512)[:, :, :Q]
        w = wp.tile([128, 3, Q], bf, tag="w", name=f"w_{h}_{b}",
                    bufs=int(_env("W_BUFS", "9")))
        if _env("EXP_SPLIT", "1") == "2":
            # two ACT ops: frees psS chunks 0-1 for head h+2 sooner
            nc.scalar.activation(w[:, 0:2, :], sview[:, 0:2, :], Exp)
            nc.scalar.activation(w[:, 2, :], sview[:, 2, :], Exp)
        else:
            nc.scalar.activation(w[:], sview, Exp)
        st[f"w{h}"] = w

    def emit_AVpair(b, P):
        st = states[b]
        vv, wa_all, waA = st["vv"], st["wa_all"], st["waA"]
        psW = sp.tile([128, 512], f32, tag="sm", name=f"psW{P}_{b}")
        for p in range(2):
            h = 2 * P + p
            w = st.pop(f"w{h}")
            for kc in range(3):
                nc.tensor.matmul(
                    psW[64 * p : 64 * p + 64, :Q],
                    vv[:, kc, 33 * h : 33 * h + 64],
                    w[:, kc, :],
                    start=(kc == 0),
                    stop=(kc == 2),
                )
        _ev = wa_ev if P < 2 else (
            nc.scalar if _env("WA_EV_LATE", "dve") == "act" else wa_ev
        )
        ev_inst = (
            _ev.copy(wa_all[:, P, :], psW[:, :Q])
            if _ev is nc.scalar
            else _ev.tensor_copy(wa_all[:, P, :], psW[:, :Q])
        )
        st["wa_evs"].append(ev_inst)
        # heads (2P, 2P+1) -> waA[j] rows 64*(P%2) .. +64
        # (contiguous partition ranges both sides: tracker-visible)
        jj = P // 2
        r0 = 64 * (P % 2)
        nc.sync.dma_start(waA[jj][r0 : r0 + 32, :], wa_all[0:32, P, :])
        nc.sync.dma_start(waA[jj][r0 + 32 : r0 + 64, :], wa_all[64:96, P, :])

    def emit_sums(b, half):
        # two DMAs gather the 4 denominator rows for pairs (2*half, 2*half+1):
        # psW row 32 holds head 2P, row 96 head 2P+1.  Contiguous partition
        # ranges on both sides keep the Tile tracker's deps exact.  Row order
        # in sums_bf is (2P, 2P+2, 2P+1, 2P+3); ind compensates.
        st = states[b]
        sb = st["sums_bf"][half]
        wa = st["wa_all"]
        nc.sync.dma_start(sb[0:2, :], wa[32:33, 2 * half : 2 * half + 2, :])
        nc.sync.dma_start(sb[2:4, :], wa[96:97, 2 * half : 2 * half + 2, :])

    def tail_chain(b, half):
        import contextlib
        hp = tc.high_priority(int(_env("CHAIN_PRI", "200"))) if _env(
            "CHAIN_HIPRI", "1"
        ) == "1" else contextlib.nullcontext()
        with hp:
            _tail_chain(b, half)

    def _tail_chain(b, half):
        st = states[b]
        sums_f = gp.tile([4, Q], f32, tag=f"sums_f{half}", name=f"sums_f{half}_{b}")
        nc.vector.tensor_copy(sums_f[:], st["sums_bf"][half][:])
        rec = gp.tile([4, Q], f32, tag=f"rec{half}", name=f"rec{half}_{b}")
        nc.vector.reciprocal_approx_fast(rec[:], sums_f[:])
        if _env("RECB_F32R", "0") == "1":
            st["recb"][half] = rec.bitcast(mybir.dt.float32r)
        else:
            recb = gp.tile([4, Q], bf, tag=f"recb{half}", name=f"recb{half}_{b}")
            nc.vector.tensor_copy(recb[:], rec[:])
            st["recb"][half] = recb

    def tail_gate(b, j):
        st = states[b]
        psR = sp.tile([128, 512], f32, tag="sm", name=f"psR{j}_{b}")
        nc.tensor.matmul(
            psR[:, :Q],
            ind_sb[:],
            st["recb"][j][:],
            start=True,
            stop=True,
        )
        g2 = gp.tile([128, Q], bf, tag="g2", name=f"g2{j}_{b}")
        nc.vector.scalar_tensor_tensor(
            g2[:], st["gt"][:, j, :], 1.0, psR[:, :Q], op0=ADD, op1=MUL
        )
        ga = gp.tile([128, Q], bf, tag="ga", name=f"ga{j}_{b}")
        ga_eng = nc.gpsimd if _env("GA_ENG", "dve") == "gps" else nc.vector
        ga_eng.tensor_tensor(ga[:], st["waA"][j][:], g2[:], op=MUL)
        st["ga"][j] = ga

    def tail_out_mm(b, qc):
        # qc 0..1 share one psum bank (two 256-wide accumulation groups);
        # qc==2 gets its own.  Evacuation happens per 256-wide group when the
        # group stops, keeping bank occupancy short.
        st = states[b]
        if st["ob"] is None:
            st["ob"] = outp.tile([128, 3, OUT], bf, tag="ob", name=f"ob_{b}")
        if qc in (0, 2):
            st["psO"] = sp.tile([128, 512], f32, tag="sm", name=f"psO{qc}_{b}")
        psO = st["psO"]
        off = OUT * (qc % 2)
        for j in range(2):
            nc.tensor.matmul(
                psO[:, off : off + OUT],
                st["ga"][j][:, 128 * qc : 128 * (qc + 1)],
                w_sb["wo"][:, j, :],
                start=(j == 0),
                stop=False,
            )
        # rank-1 accumulate of o_bias (fp8 hi/lo DoubleRow)
        nc.tensor.matmul(
            psO[:, off : off + OUT],
            ones2[:],
            obias2[:],
            start=False,
            stop=True,
            perf_mode=DR,
        )
        ecopy(ob_ev, st["ob"][:, qc, :], psO[:, off : off + OUT])

    def tail_out(b):
        st = states[b]
        nc.sync.dma_start(
            io["out"][b].rearrange("(c p) o -> p c o", p=128), st["ob"][:]
        )

    # ---- slot-woven software pipeline ----
    # slot h of batch b emits: S/exp/mul for head h of b, lagged AV pairs of
    # b (with the first denominator half's recip chain still inside b), the
    # remaining tail of b-1, and proj piece h of b+1.
    emit_loads(0)
    if bpc > 1:
        emit_loads(1)
    for i in range(8):
        PROJ_PIECES[i](0)
    for b in range(bpc):
        for h in range(9):
            if h < 8:
                emit_S(b, h)
            if h == 2:
                emit_AVpair(b, 0)
            elif h == 4:
                emit_AVpair(b, 1)
                emit_sums(b, 0)
            elif h == 6:
                tail_chain(b, 0)
            elif h == 7:
                emit_AVpair(b, 2)
            elif h == 8:
                emit_AVpair(b, 3)
                emit_sums(b, 1)
            if b > 0:
                pb = b - 1
                if h == 0:
                    tail_chain(pb, 1)
                elif h == 1:
                    tail_gate(pb, 0)
                elif h == 2:
                    tail_gate(pb, 1)
                elif h == 3:
                    tail_out_mm(pb, 0)
                elif h == 4:
                    tail_out_mm(pb, 1)
                elif h == 5:
                    tail_out_mm(pb, 2)
                elif h == 6:
                    tail_out(pb)
            if h < 8 and b + 1 < bpc:
                PROJ_PIECES[h](b + 1)
            if h == 2 and b + 2 < bpc:
                emit_loads(b + 2)
        if b - 1 >= 0:
            states.pop(b - 1)
    bl = bpc - 1
    tail_chain(bl, 1)
    tail_gate(bl, 0)
    tail_gate(bl, 1)
    for qc in range(3):
        tail_out_mm(bl, qc)
    tail_out(bl)


def build(bpc=BPC):
    nc = bacc.Bacc(
        "TRN2",
        target_bir_lowering=False,
        debug=False,
        enable_asserts=False,
        num_devices=NUM_CORES,
    )
    f32, bf = mybir.dt.float32, mybir.dt.bfloat16
    fp8 = mybir.dt.float8e4
    io = {
        "inT": nc.dram_tensor("inT", [bpc, 128, 4, Q], bf, kind="ExternalInput").ap(),
        "cbT": nc.dram_tensor(
            "cbT", [bpc, 128, H, 3, 2, Q], fp8, kind="ExternalInput"
        ).ap(),
        "idup": nc.dram_tensor("idup", [128, 2, 128], fp8, kind="ExternalInput").ap(),
        "wq": nc.dram_tensor("wq", [128, 2, 256], bf, kind="ExternalInput").ap(),
        "wk": nc.dram_tensor("wk", [128, 2, 256], bf, kind="ExternalInput").ap(),
        "wv": nc.dram_tensor("wv", [128, 2, 256], bf, kind="ExternalInput").ap(),
        "wg": nc.dram_tensor("wg", [128, 2, 256], bf, kind="ExternalInput").ap(),
        "wo": nc.dram_tensor("wo", [128, 2, 256], bf, kind="ExternalInput").ap(),
        "ones2": nc.dram_tensor("ones2", [1, 2, 128], fp8, kind="ExternalInput").ap(),
        "obias2": nc.dram_tensor("obias2", [1, 2, OUT], fp8, kind="ExternalInput").ap(),
        "gbh": nc.dram_tensor("gbh", [128, 2], f32, kind="ExternalInput").ap(),
        "ind": nc.dram_tensor("ind", [4, 128], bf, kind="ExternalInput").ap(),
        "out": nc.dram_tensor("out", [bpc, Q, OUT], bf, kind="ExternalOutput").ap(),
    }
    with tile.TileContext(nc) as tc:
        with ExitStack() as ctx:
            _build_body(ctx, tc, io, bpc)
    nc.compile()
    return nc


def _prep_inputs(
    q_data,
    m_data,
    bias,
    nonbatched_bias,
    q_weights,
    k_weights,
    v_weights,
    o_weights,
    o_bias,
    gating_w,
    gating_b,
):
    """Host-side preprocessing into the DMA-friendly device layouts."""
    scale = q_weights.shape[-1] ** -0.5

    def featT(x):  # [B, S, A] -> [B, 128, A//128, S]
        b, s, a = x.shape
        t = x.transpose(0, 2, 1).reshape(b, a // 128, 128, s).transpose(0, 2, 1, 3)
        return np.ascontiguousarray(t.astype(BF16))

    qdT = featT(q_data)
    mdT = featT(m_data)
    inT = np.ascontiguousarray(np.concatenate([qdT, mdT], axis=2))

    # cb[b, p, h, kc, t, q] = hi/lo e4m3 split of
    #   bias[b, q, 128*kc+p] + nb[h, q, 128*kc+p]
    bT = bias[:, 0].transpose(0, 2, 1).astype(np.float32)  # [B, K, Q]
    nT = nonbatched_bias.transpose(0, 2, 1).astype(np.float32)  # [H, K, Q]
    comb = bT[:, None] + nT[None]  # [B, H, K, Q]
    hi = comb.astype(E4M3)
    lo = (comb - hi.astype(np.float32)).astype(E4M3)
    cbT = np.stack([hi, lo], axis=-2)  # [B, H, K, 2, Q]
    cbT = np.ascontiguousarray(
        cbT.reshape(B, H, 3, 128, 2, Q).transpose(0, 3, 1, 2, 4, 5)
    )
    idup = np.zeros((128, 2, 128), dtype=E4M3)
    for t in range(2):
        np.fill_diagonal(idup[:, t, :], 1.0)

    def wmat(w, s=1.0):  # [A, H, hd] -> [128, 2, 256]
        m = (w.reshape(A, H * HD) * s).astype(BF16)
        return np.ascontiguousarray(m.reshape(2, 128, 256).transpose(1, 0, 2))

    wq = wmat(q_weights, scale)
    wk = wmat(k_weights)
    wv = wmat(v_weights)
    wg = wmat(gating_w)
    wo = np.ascontiguousarray(
        o_weights.reshape(256, 256).astype(BF16).reshape(2, 128, 256).transpose(1, 0, 2)
    )
    # o_bias rank-1 via fp8 hi/lo DoubleRow: ones2.T @ obias2 = 16*(hi+lo)/16
    # scale bias up x16 before fp8 split, ones row = 1/16 (exact in fp8)
    obf = o_bias.astype(np.float32) * 16.0
    hi = np.clip(obf, -240, 240).astype(E4M3)
    lo = np.clip(obf - hi.astype(np.float32), -240, 240).astype(E4M3)
    obias2 = np.ascontiguousarray(np.stack([hi, lo]).reshape(1, 2, OUT))
    ones2 = np.full((1, 2, 128), 1.0 / 16.0, dtype=E4M3)
    gbh = np.ascontiguousarray(
        (0.5 * gating_b.reshape(H * HD).astype(np.float32)).reshape(2, 128).T
    )
    # indicator for the recip broadcast: sums rows are pair-major = head
    # order within each j-group, so one [4,128] block serves both j's
    ind = np.zeros((4, 128), dtype=BF16)
    for r, hh in enumerate((0, 2, 1, 3)):
        ind[r, 32 * hh : 32 * (hh + 1)] = 1.0
    return dict(
        inT=inT, cbT=cbT, wq=wq, wk=wk, wv=wv, wg=wg, wo=wo,
        ones2=ones2, obias2=obias2, gbh=gbh, ind=ind, idup=idup,
    )


_NC_CACHE = {}


def kernel(**inputs):
    from concourse.bass_utils import run_bass_kernel_spmd

    full = _prep_inputs(**{k: np.asarray(v) for k, v in inputs.items()})
    if BPC not in _NC_CACHE:
        _NC_CACHE[BPC] = build(BPC)
    nc = _NC_CACHE[BPC]

    shared = {
        k: full[k]
        for k in ("wq", "wk", "wv", "wg", "wo", "ones2", "obias2", "gbh", "ind", "idup")
    }
    in_maps = []
    for c in range(NUM_CORES):
        sl = slice(c * BPC, (c + 1) * BPC)
        in_maps.append(dict(inT=full["inT"][sl], cbT=full["cbT"][sl], **shared))

    trace = bool(int(os.environ.get("BASS_KERNEL_TRACE", "0")))
    if trace:
        try:
            from antenv.axon_hooks import get_axon_ntff_profile_hook  # noqa: F401
        except Exception:
            trace = False
    import time

    t0 = time.time()
    res = run_bass_kernel_spmd(
        nc, in_maps, core_ids=list(range(NUM_CORES)), trace=trace
    )
    kernel.last_run_wall_s = time.time() - t0
    if trace and res.exec_time_ns is not None:
        print(f"HW exec time: {res.exec_time_ns} ns")
        kernel.last_exec_time_ns = res.exec_time_ns
    out = np.concatenate([r["out"] for r in res.results], axis=0)
    return out.astype(np.float32)


# revision 49
# speedup vs baseline: 1.0187x; 1.0187x over previous
"""AlphaFold-style gated MSA attention on 8 Trainium2 NeuronCores.

Batch-sharded (128 batches -> 16 per core). Full inputs in, full output out.

Math per batch b (reference):
  q = (q_data @ Wq) * hk^-0.5          [Q, H, 32]
  k = m_data @ Wk ; v = m_data @ Wv    [K, H, 32]
  S[h] = q_h k_h^T + bias[b] + nb[h]   [H, Q, K]
  w = softmax(S, axis=-1)
  wa = w @ v                            [Q, H, 32]
  gate = sigmoid(q_data @ Wg + gb)
  out = (wa * gate).reshape(Q, 256) @ Wo + o_bias

Device-side formulation (per core, per batch):
  All tensors transposed (feature dim on partitions).  S^T[k, q] per head
  from kT/qT projections.  Softmax is unnormalized: the host precomputes
  pw[b, h] = exp(bias[b] + nb[h])^T in bf16 (one DMA per batch) so the
  per-head weights are a single DVE multiply:
      w^T = exp(S^T) * pw[h]
  The V-matmul uses lhsT = [v_h | 2.0] so PSUM row 32/96 accumulates
  2*sum_k w per head (softmax denominators for free).  Head pairs share a
  PSUM bank; evacuated pairs land in wa_all[:, pair, :], from which one
  strided DMA per batch gathers all 8 denominator rows and four per-pair
  DMAs rearrange heads into j-layout waA tiles for the output projection.
  Normalization and gating fuse:
      ga^T = waA * (1 + tanh(x/2 + gb/2)) * recip(2*sum)
  with the per-head recip broadcast across 32 partitions by an indicator
  matmul.  Output projection back to [q, 256] with o_bias added via a
  rank-1 fp8 hi/lo DoubleRow matmul; output stored bf16.
"""

import os
import sys

sys.path.insert(0, "/opt/trn_rl_repo")

import numpy as np
import ml_dtypes
from contextlib import ExitStack

import concourse.bass as bass  # noqa: F401  (engine types)
import concourse.bacc as bacc
import concourse.mybir as mybir
import concourse.tile as tile

BF16 = ml_dtypes.bfloat16
E4M3 = ml_dtypes.float8_e4m3fn

NUM_CORES = 8
B, Q, K, A = 128, 384, 384, 256
H, HD = 8, 32  # heads, head dim
OUT = 256
BPC = B // NUM_CORES  # batches per core


def _env(name, default):
    return os.environ.get(name, default)


def _build_body(ctx, tc, io, bpc):
    nc = tc.nc
    f32, bf = mybir.dt.float32, mybir.dt.bfloat16
    fp8 = mybir.dt.float8e4
    Exp = mybir.ActivationFunctionType.Exp
    Tanh = mybir.ActivationFunctionType.Tanh
    MUL, ADD = mybir.AluOpType.mult, mybir.AluOpType.add
    DR = mybir.MatmulPerfMode.DoubleRow

    const = ctx.enter_context(tc.tile_pool(name="const", bufs=1))
    lp = ctx.enter_context(tc.tile_pool(name="loads", bufs=int(_env("LP_BUFS", "3"))))
    pwp = ctx.enter_context(tc.tile_pool(name="pw", bufs=int(_env("PW_BUFS", "2"))))
    pp = ctx.enter_context(tc.tile_pool(name="proj", bufs=int(_env("PP_BUFS", "4"))))
    wp = ctx.enter_context(tc.tile_pool(name="work", bufs=int(_env("WP_BUFS", "4"))))
    wap = ctx.enter_context(tc.tile_pool(name="wa", bufs=int(_env("WA_BUFS", "2"))))
    gp = ctx.enter_context(tc.tile_pool(name="gating", bufs=int(_env("GP_BUFS", "3"))))
    outp = ctx.enter_context(tc.tile_pool(name="outp", bufs=3))
    # PSUM: 2 x 3 banks (S^T) + 2 x 1 bank (everything else) = 8 banks.
    Sp = ctx.enter_context(tc.tile_pool(name="psum_S", bufs=int(_env("S_BUFS", "2")), space="PSUM"))
    sp = ctx.enter_context(tc.tile_pool(name="psum_sm", bufs=int(_env("SP_BUFS", "2")), space="PSUM"))

    # ---- resident constants ----
    wall = const.tile([128, 5, 2, 256], bf, tag="wall", name="wall")
    nc.sync.dma_start(wall[:], io["wall"])
    w_sb = {
        name: wall[:, i] for i, name in enumerate(("wq", "wk", "wv", "wg", "wo"))
    }
    if _env("ACT_WARM", "1") == "1":
        # dummy activation right after the packed-weights DMA: pulls the
        # ~1.3us ACT table load off the critical path (exp_and_others holds
        # both Exp and Tanh, so no further loads fire later)
        warm = const.tile([128, 2], f32, tag="warm")
        nc.scalar.activation(warm[:], wall[:, 0, 0, 0:2], Exp)
    # small constants: tiles declared here, DMAs deferred until after the
    # first batches' input loads have claimed the HWDGE (emit_small_consts)
    ones2 = const.tile([1, 2, 128], fp8, tag="ones2")
    obias2 = const.tile([1, 2, OUT], fp8, tag="obias2")
    gbh_sb = const.tile([128, 2], f32, tag="gbh")
    ind_sb = const.tile([4, 128], bf, tag="ind")
    idup = const.tile([128, 2, 128], fp8, tag="idup")

    def emit_small_consts():
        nc.sync.dma_start(ones2[:], io["ones2"])
        nc.sync.dma_start(obias2[:], io["obias2"])
        nc.sync.dma_start(gbh_sb[:], io["gbh"])
        nc.sync.dma_start(ind_sb[:], io["ind"])
        nc.sync.dma_start(idup[:], io["idup"])

    # GPSIMD has no PSUM port: all psum evacuations go ACT or DVE.
    qk_ev = nc.scalar if _env("QK_EV", "dve") == "act" else nc.vector
    vv_ev = nc.scalar if _env("VV_EV", "dve") == "act" else nc.vector
    ob_ev = nc.scalar if _env("OB_EV", "dve") == "act" else nc.vector
    wa_ev = nc.scalar if _env("WA_EV", "dve") == "act" else nc.vector

    def ecopy(eng, dst, src):
        if eng is nc.scalar:
            eng.copy(dst, src)
        else:
            eng.tensor_copy(dst, src)

    from concourse.tile_rust import add_dep_helper as _adh

    wa_war = {}  # buffer-slot -> last strided sums DMA reading that slot
    n_wa_bufs = int(_env("WA_BUFS", "2"))
    states = {}

    def emit_loads(b, ld_pri=None):
        import contextlib
        hp = tc.high_priority() if ld_pri == 0 else contextlib.nullcontext()
        ld = lp.tile([128, 4, Q], bf, tag="ld", name=f"ld_{b}")
        with hp:
            nc.sync.dma_start(ld[:], io["inT"][b])
        cb = pwp.tile([128, H, 3, 2, Q], fp8, tag="cb", name=f"cb_{b}")
        nc.sync.dma_start(cb[:], io["cbT"][b])
        qT = pp.tile([128, 2, Q], bf, tag="qT", name=f"qT_{b}")  # [hc, j, q]
        kT = pp.tile([128, 2, Q], bf, tag="kT", name=f"kT_{b}")  # [hc, j, k]
        gt = pp.tile([128, 2, Q], bf, tag="gt", name=f"gt_{b}")
        # [k, kc, h*33+c | 2.0]; padded to 320 so every head has a 64-wide
        # lhsT window (M=64 writes initialized junk to PSUM rows 32-63,
        # letting the pair evacuation be one full-width copy).
        vv = pp.tile([128, 3, 320], bf, tag="vv", name=f"vv_{b}")
        sums_bf = [
            gp.tile([4, Q], bf, tag=f"sums_bf{i}", name=f"sums_bf{i}_{b}")
            for i in range(2)
        ]
        wa_all = wap.tile([128, 4, Q], bf, tag="wa_all", name=f"wa_all_{b}")
        waA = [
            pp.tile([128, Q], bf, tag=f"waA{j}", name=f"waA{j}_{b}") for j in range(2)
        ]
        states[b] = dict(
            ld=ld, cb=cb, qT=qT, kT=kT, gt=gt, vv=vv,
            sums_bf=sums_bf, wa_all=wa_all, waA=waA, psW=None, psO=None,
            wa_evs=[], ga=[None, None], ob=None, recb=[None, None],
            recb_tc=[None, None],
        )

    def proj_qkg(b, which, j):
        # which: "q" -> qT, "k" -> kT, "g" -> gt (tanh)
        st = states[b]
        ld = st["ld"]
        src_, wname = (ld[:, 0:2, :], "wq") if which == "q" else (
            (ld[:, 2:4, :], "wk") if which == "k" else (ld[:, 0:2, :], "wg")
        )
        ps = sp.tile([128, 512], f32, tag="sm", name=f"ps{which}{j}_{b}")
        for a in range(2):
            nc.tensor.matmul(
                ps[:, :Q],
                w_sb[wname][:, a, 128 * j : 128 * (j + 1)],
                src_[:, a, :],
                start=(a == 0),
                stop=(a == 1),
            )
        if which == "g":
            nc.scalar.activation(
                st["gt"][:, j, :], ps[:, :Q], Tanh, bias=gbh_sb[:, j : j + 1], scale=0.5
            )
        else:
            dst = st["qT"] if which == "q" else st["kT"]
            ecopy(qk_ev, dst[:, j, :], ps[:, :Q])

    def proj_v01(b):
        st = states[b]
        md = st["ld"][:, 2:4, :]
        vv = st["vv"]
        nc.gpsimd.memset(vv[:], 2.0)
        ps = sp.tile([128, 512], f32, tag="sm", name=f"psv01_{b}")
        for kc in range(2):
            for a in range(2):
                nc.tensor.matmul(
                    ps[:, 256 * kc : 256 * (kc + 1)],
                    md[:, a, 128 * kc : 128 * (kc + 1)],
                    w_sb["wv"][:, a, :],
                    start=(a == 0),
                    stop=(a == 1),
                )
        ecopy(
            vv_ev,
            vv[:, 0:2, 0:264].rearrange("p k (h c) -> p k h c", c=33)[:, :, :, 0:32],
            ps[:].rearrange("p (k h c) -> p k h c", k=2, c=32),
        )

    def proj_v2(b):
        st = states[b]
        md = st["ld"][:, 2:4, :]
        ps = sp.tile([128, 512], f32, tag="sm", name=f"psv2_{b}")
        for a in range(2):
            nc.tensor.matmul(
                ps[:, :256],
                md[:, a, 256:384],
                w_sb["wv"][:, a, :],
                start=(a == 0),
                stop=(a == 1),
            )
        ecopy(
            vv_ev,
            st["vv"][:, 2, 0:264].rearrange("p (h c) -> p h c", c=33)[:, :, 0:32],
            ps[:, :256].rearrange("p (h c) -> p h c", c=32),
        )

    PROJ_PIECES = [
        lambda b: proj_qkg(b, "q", 0),
        lambda b: proj_qkg(b, "k", 0),
        lambda b: proj_qkg(b, "q", 1),
        lambda b: proj_qkg(b, "k", 1),
        proj_v01,
        proj_v2,
        lambda b: proj_qkg(b, "g", 0),
        lambda b: proj_qkg(b, "g", 1),
    ]

    def emit_S(b, h):
        st = states[b]
        j, hh = h // 4, h % 4
        psS = Sp.tile([128, 1536], f32, tag="S", name=f"psS{h}_{b}")
        for kc in range(3):
            qk_mm = nc.tensor.matmul(
                psS[:, 512 * kc : 512 * kc + Q],
                st["kT"][32 * hh : 32 * (hh + 1), j, 128 * kc : 128 * (kc + 1)],
                st["qT"][32 * hh : 32 * (hh + 1), j, :],
                start=True,
                stop=False,
                tile_position=(32 * hh, 0),
            )
            # accumulate bias+nb via fp8 hi/lo DoubleRow identity-add
            add_mm = nc.tensor.matmul(
                psS[:, 512 * kc : 512 * kc + Q],
                idup[:],
                st["cb"][:, h, kc, :, :],
                start=False,
                stop=True,
                perf_mode=DR,
            )
            # no data dep links the group members; forbid scheduler reorder
            _adh(add_mm.ins, qk_mm.ins, reason="accumulation group order")
        sview = psS[:].rearrange("p (c x) -> p c x", x=512)[:, :, :Q]
        w = wp.tile([128, 3, Q], bf, tag="w", name=f"w_{h}_{b}",
                    bufs=int(_env("W_BUFS", "9")))
        if _env("EXP_SPLIT", "1") == "2":
            # two ACT ops: frees psS chunks 0-1 for head h+2 sooner
            nc.scalar.activation(w[:, 0:2, :], sview[:, 0:2, :], Exp)
            nc.scalar.activation(w[:, 2, :], sview[:, 2, :], Exp)
        else:
            nc.scalar.activation(w[:], sview, Exp)
        st[f"w{h}"] = w

    def emit_AVpair(b, P):
        st = states[b]
        vv, wa_all, waA = st["vv"], st["wa_all"], st["waA"]
        psW = sp.tile([128, 512], f32, tag="sm", name=f"psW{P}_{b}")
        for p in range(2):
            h = 2 * P + p
            w = st.pop(f"w{h}")
            for kc in range(3):
                nc.tensor.matmul(
                    psW[64 * p : 64 * p + 64, :Q],
                    vv[:, kc, 33 * h : 33 * h + 64],
                    w[:, kc, :],
                    start=(kc == 0),
                    stop=(kc == 2),
                )
        _ev = wa_ev if P < 2 else (
            nc.scalar if _env("WA_EV_LATE", "dve") == "act" else wa_ev
        )
        ev_inst = (
            _ev.copy(wa_all[:, P, :], psW[:, :Q])
            if _ev is nc.scalar
            else _ev.tensor_copy(wa_all[:, P, :], psW[:, :Q])
        )
        st["wa_evs"].append(ev_inst)
        # heads (2P, 2P+1) -> waA[j] rows 64*(P%2) .. +64
        # (contiguous partition ranges both sides: tracker-visible)
        jj = P // 2
        r0 = 64 * (P % 2)
        nc.sync.dma_start(waA[jj][r0 : r0 + 32, :], wa_all[0:32, P, :])
        nc.sync.dma_start(waA[jj][r0 + 32 : r0 + 64, :], wa_all[64:96, P, :])

    def emit_sums(b, half):
        # two DMAs gather the 4 denominator rows for pairs (2*half, 2*half+1):
        # psW row 32 holds head 2P, row 96 head 2P+1.  Contiguous partition
        # ranges on both sides keep the Tile tracker's deps exact.  Row order
        # in sums_bf is (2P, 2P+2, 2P+1, 2P+3); ind compensates.
        st = states[b]
        sb = st["sums_bf"][half]
        wa = st["wa_all"]
        nc.sync.dma_start(sb[0:2, :], wa[32:33, 2 * half : 2 * half + 2, :])
        nc.sync.dma_start(sb[2:4, :], wa[96:97, 2 * half : 2 * half + 2, :])

    def tail_chain(b, half):
        import contextlib
        hp = tc.high_priority(int(_env("CHAIN_PRI", "200"))) if _env(
            "CHAIN_HIPRI", "1"
        ) == "1" else contextlib.nullcontext()
        with hp:
            _tail_chain(b, half)

    def _tail_chain(b, half):
        st = states[b]
        sums_f = gp.tile([4, Q], f32, tag=f"sums_f{half}", name=f"sums_f{half}_{b}")
        nc.vector.tensor_copy(sums_f[:], st["sums_bf"][half][:])
        rec = gp.tile([4, Q], f32, tag=f"rec{half}", name=f"rec{half}_{b}")
        nc.vector.reciprocal_approx_fast(rec[:], sums_f[:])
        if _env("RECB_F32R", "0") == "1":
            st["recb"][half] = rec.bitcast(mybir.dt.float32r)
        else:
            recb = gp.tile([4, Q], bf, tag=f"recb{half}", name=f"recb{half}_{b}")
            nc.vector.tensor_copy(recb[:], rec[:])
            st["recb"][half] = recb

    def tail_gate(b, j):
        st = states[b]
        psR = sp.tile([128, 512], f32, tag="sm", name=f"psR{j}_{b}")
        nc.tensor.matmul(
            psR[:, :Q],
            ind_sb[:],
            st["recb"][j][:],
            start=True,
            stop=True,
        )
        g2 = gp.tile([128, Q], bf, tag="g2", name=f"g2{j}_{b}")
        nc.vector.scalar_tensor_tensor(
            g2[:], st["gt"][:, j, :], 1.0, psR[:, :Q], op0=ADD, op1=MUL
        )
        ga = gp.tile([128, Q], bf, tag="ga", name=f"ga{j}_{b}")
        ga_eng = nc.gpsimd if _env("GA_ENG", "dve") == "gps" else nc.vector
        ga_eng.tensor_tensor(ga[:], st["waA"][j][:], g2[:], op=MUL)
        st["ga"][j] = ga

    def tail_out_mm(b, qc):
        # qc 0..1 share one psum bank (two 256-wide accumulation groups);
        # qc==2 gets its own.  Evacuation happens per 256-wide group when the
        # group stops, keeping bank occupancy short.
        st = states[b]
        if st["ob"] is None:
            st["ob"] = outp.tile([128, 3, OUT], bf, tag="ob", name=f"ob_{b}")
        if qc in (0, 2):
            st["psO"] = sp.tile([128, 512], f32, tag="sm", name=f"psO{qc}_{b}")
        psO = st["psO"]
        off = OUT * (qc % 2)
        for j in range(2):
            nc.tensor.matmul(
                psO[:, off : off + OUT],
                st["ga"][j][:, 128 * qc : 128 * (qc + 1)],
                w_sb["wo"][:, j, :],
                start=(j == 0),
                stop=False,
            )
        # rank-1 accumulate of o_bias (fp8 hi/lo DoubleRow)
        nc.tensor.matmul(
            psO[:, off : off + OUT],
            ones2[:],
            obias2[:],
            start=False,
            stop=True,
            perf_mode=DR,
        )
        ecopy(ob_ev, st["ob"][:, qc, :], psO[:, off : off + OUT])

    def tail_out(b):
        st = states[b]
        nc.sync.dma_start(
            io["out"][b].rearrange("(c p) o -> p c o", p=128), st["ob"][:]
        )

    # ---- slot-woven software pipeline ----
    # slot h of batch b emits: S/exp/mul for head h of b, lagged AV pairs of
    # b (with the first denominator half's recip chain still inside b), the
    # remaining tail of b-1, and proj piece h of b+1.
    emit_loads(0)
    emit_small_consts()
    if bpc > 1:
        emit_loads(1)
    for i in range(8):
        PROJ_PIECES[i](0)
    for b in range(bpc):
        for h in range(9):
            if h < 8:
                emit_S(b, h)
            if h == 2:
                emit_AVpair(b, 0)
            elif h == 4:
                emit_AVpair(b, 1)
                emit_sums(b, 0)
            elif h == 6:
                tail_chain(b, 0)
            elif h == 7:
                emit_AVpair(b, 2)
            elif h == 8:
                emit_AVpair(b, 3)
                emit_sums(b, 1)
            if b > 0:
                pb = b - 1
                if h == 0:
                    tail_chain(pb, 1)
                elif h == 1:
                    tail_gate(pb, 0)
                elif h == 2:
                    tail_gate(pb, 1)
                elif h == 3:
                    tail_out_mm(pb, 0)
                elif h == 4:
                    tail_out_mm(pb, 1)
                elif h == 5:
                    tail_out_mm(pb, 2)
                elif h == 6:
                    tail_out(pb)
            if h < 8 and b + 1 < bpc:
                PROJ_PIECES[h](b + 1)
            if h == 2 and b + 2 < bpc:
                emit_loads(b + 2)
        if b - 1 >= 0:
            states.pop(b - 1)
    bl = bpc - 1
    tail_chain(bl, 1)
    tail_gate(bl, 0)
    tail_gate(bl, 1)
    for qc in range(3):
        tail_out_mm(bl, qc)
    tail_out(bl)


def build(bpc=BPC):
    nc = bacc.Bacc(
        "TRN2",
        target_bir_lowering=False,
        debug=False,
        enable_asserts=False,
        num_devices=NUM_CORES,
    )
    f32, bf = mybir.dt.float32, mybir.dt.bfloat16
    fp8 = mybir.dt.float8e4
    io = {
        "inT": nc.dram_tensor("inT", [bpc, 128, 4, Q], bf, kind="ExternalInput").ap(),
        "cbT": nc.dram_tensor(
            "cbT", [bpc, 128, H, 3, 2, Q], fp8, kind="ExternalInput"
        ).ap(),
        "idup": nc.dram_tensor("idup", [128, 2, 128], fp8, kind="ExternalInput").ap(),
        "wall": nc.dram_tensor(
            "wall", [128, 5, 2, 256], bf, kind="ExternalInput"
        ).ap(),
        "ones2": nc.dram_tensor("ones2", [1, 2, 128], fp8, kind="ExternalInput").ap(),
        "obias2": nc.dram_tensor("obias2", [1, 2, OUT], fp8, kind="ExternalInput").ap(),
        "gbh": nc.dram_tensor("gbh", [128, 2], f32, kind="ExternalInput").ap(),
        "ind": nc.dram_tensor("ind", [4, 128], bf, kind="ExternalInput").ap(),
        "out": nc.dram_tensor("out", [bpc, Q, OUT], bf, kind="ExternalOutput").ap(),
    }
    with tile.TileContext(nc) as tc:
        with ExitStack() as ctx:
            _build_body(ctx, tc, io, bpc)
    nc.compile()
    return nc


def _prep_inputs(
    q_data,
    m_data,
    bias,
    nonbatched_bias,
    q_weights,
    k_weights,
    v_weights,
    o_weights,
    o_bias,
    gating_w,
    gating_b,
):
    """Host-side preprocessing into the DMA-friendly device layouts."""
    scale = q_weights.shape[-1] ** -0.5

    def featT(x):  # [B, S, A] -> [B, 128, A//128, S]
        b, s, a = x.shape
        t = x.transpose(0, 2, 1).reshape(b, a // 128, 128, s).transpose(0, 2, 1, 3)
        return np.ascontiguousarray(t.astype(BF16))

    qdT = featT(q_data)
    mdT = featT(m_data)
    inT = np.ascontiguousarray(np.concatenate([qdT, mdT], axis=2))

    # cb[b, p, h, kc, t, q] = hi/lo e4m3 split of
    #   bias[b, q, 128*kc+p] + nb[h, q, 128*kc+p]
    bT = bias[:, 0].transpose(0, 2, 1).astype(np.float32)  # [B, K, Q]
    nT = nonbatched_bias.transpose(0, 2, 1).astype(np.float32)  # [H, K, Q]
    comb = bT[:, None] + nT[None]  # [B, H, K, Q]
    hi = comb.astype(E4M3)
    lo = (comb - hi.astype(np.float32)).astype(E4M3)
    cbT = np.stack([hi, lo], axis=-2)  # [B, H, K, 2, Q]
    cbT = np.ascontiguousarray(
        cbT.reshape(B, H, 3, 128, 2, Q).transpose(0, 3, 1, 2, 4, 5)
    )
    idup = np.zeros((128, 2, 128), dtype=E4M3)
    for t in range(2):
        np.fill_diagonal(idup[:, t, :], 1.0)

    def wmat(w, s=1.0):  # [A, H, hd] -> [128, 2, 256]
        m = (w.reshape(A, H * HD) * s).astype(BF16)
        return np.ascontiguousarray(m.reshape(2, 128, 256).transpose(1, 0, 2))

    wq = wmat(q_weights, scale)
    wk = wmat(k_weights)
    wv = wmat(v_weights)
    wg = wmat(gating_w)
    wo = np.ascontiguousarray(
        o_weights.reshape(256, 256).astype(BF16).reshape(2, 128, 256).transpose(1, 0, 2)
    )
    wall = np.ascontiguousarray(np.stack([wq, wk, wv, wg, wo], axis=1))
    # o_bias rank-1 via fp8 hi/lo DoubleRow: ones2.T @ obias2 = 16*(hi+lo)/16
    # scale bias up x16 before fp8 split, ones row = 1/16 (exact in fp8)
    obf = o_bias.astype(np.float32) * 16.0
    hi = np.clip(obf, -240, 240).astype(E4M3)
    lo = np.clip(obf - hi.astype(np.float32), -240, 240).astype(E4M3)
    obias2 = np.ascontiguousarray(np.stack([hi, lo]).reshape(1, 2, OUT))
    ones2 = np.full((1, 2, 128), 1.0 / 16.0, dtype=E4M3)
    gbh = np.ascontiguousarray(
        (0.5 * gating_b.reshape(H * HD).astype(np.float32)).reshape(2, 128).T
    )
    # indicator for the recip broadcast: sums rows are pair-major = head
    # order within each j-group, so one [4,128] block serves both j's
    ind = np.zeros((4, 128), dtype=BF16)
    for r, hh in enumerate((0, 2, 1, 3)):
        ind[r, 32 * hh : 32 * (hh + 1)] = 1.0
    return dict(
        inT=inT, cbT=cbT, wall=wall,
        ones2=ones2, obias2=obias2, gbh=gbh, ind=ind, idup=idup,
    )


_NC_CACHE = {}


def kernel(**inputs):
    from concourse.bass_utils import run_bass_kernel_spmd

    full = _prep_inputs(**{k: np.asarray(v) for k, v in inputs.items()})
    if BPC not in _NC_CACHE:
        _NC_CACHE[BPC] = build(BPC)
    nc = _NC_CACHE[BPC]

    shared = {
        k: full[k]
        for k in ("wall", "ones2", "obias2", "gbh", "ind", "idup")
    }
    in_maps = []
    for c in range(NUM_CORES):
        sl = slice(c * BPC, (c + 1) * BPC)
        in_maps.append(dict(inT=full["inT"][sl], cbT=full["cbT"][sl], **shared))

    trace = bool(int(os.environ.get("BASS_KERNEL_TRACE", "0")))
    if trace:
        try:
            from antenv.axon_hooks import get_axon_ntff_profile_hook  # noqa: F401
        except Exception:
            trace = False
    import time

    t0 = time.time()
    res = run_bass_kernel_spmd(
        nc, in_maps, core_ids=list(range(NUM_CORES)), trace=trace
    )
    kernel.last_run_wall_s = time.time() - t0
    if trace and res.exec_time_ns is not None:
        print(f"HW exec time: {res.exec_time_ns} ns")
        kernel.last_exec_time_ns = res.exec_time_ns
    out = np.concatenate([r["out"] for r in res.results], axis=0)
    return out.astype(np.float32)
